# revision 1
# baseline (speedup 1.0000x reference)
"""Trainium2 Bass kernel for nn_ContextualModel_75806172774985.

Per-sample computation (B = 4M samples, S=4 steps, Q=5 features):
    y[b, m] = sum_{s < L[b]} q0[b,s] * (A @ feats[b,s])[m],
    A = W_reg @ W_kernel  (4x4)

Sharding: pure data parallel over 8 NeuronCores, batch split 500k/core,
zero-padded to 507904 = 128 partitions x 3968 samples; each partition owns a
contiguous run of samples; tiles of K samples/partition.

Engine split per tile:
    VectorE : 4x scalar_tensor_tensor  Z_s = (L > s) * q0_s  (fused mask)
              2x tensor_mul  M_s[k,f] = Z_s[k] * x[k,s,f]    (s = 0, 1)
    GpSimd  : 2x tensor_mul  (s = 2, 3)
    TensorE : per 512-col group (128 samples/partition):
                4x identity-matmul accumulate (fp32r) -> v in PSUM
                4x transpose (fp32) -> vT
                1x block-diag-W matmul (fp32r) -> y' = A @ v per sample
                4x transpose (fp32) -> y'' back in sample-major layout
              W = blockdiag_32(A) built on-device: PE outer-product of
              replicated weights, masked by an inline kron(I32, ones(4,4)).
    ScalarE : 3 PSUM->SBUF staging copies per group
    SyncE   : HWDGE DMAs; y'' is DMA'd to HBM straight from PSUM
"""
import numpy as np

import concourse.bass as bass
import concourse.tile as tile
from concourse import bacc, mybir
from concourse.bass_utils import run_bass_kernel_spmd

N_CORES = 8
P = 128
B_TOTAL = 4_000_000
BS = B_TOTAL // N_CORES          # 500_000 samples per core

f32 = mybir.dt.float32
f32r = mybir.dt.float32r
i32 = mybir.dt.int32

GROUP = 512                      # v-columns per PE group (= 128 samples)


def build_nc(k_tiles=(640, 640, 640, 640, 640, 640, 128), num_devices=N_CORES):
    """Build the Bass program. k_tiles: samples/partition for each tile;
    every K must be a multiple of 128 (so tiles split into whole 512-col
    PE groups)."""
    for k in k_tiles:
        assert k % 128 == 0
    T = sum(k_tiles)             # samples per partition
    bs_pad = P * T

    nc = bacc.Bacc("TRN2", target_bir_lowering=False, debug=False,
                   enable_asserts=False, num_devices=num_devices)

    x_d = nc.dram_tensor("xss", [bs_pad, 20], f32, kind="ExternalInput")
    l_d = nc.dram_tensor("seq", [bs_pad], i32, kind="ExternalInput")
    wk_d = nc.dram_tensor("w_kernel", [4, 4], f32, kind="ExternalInput")
    wr_d = nc.dram_tensor("w_reg", [4, 4], f32, kind="ExternalInput")
    y_d = nc.dram_tensor("y", [bs_pad, 4], f32, kind="ExternalOutput")

    # constants embedded in the NEFF (input-independent)
    ident_np = np.eye(128, dtype=np.float32)
    ident_d = nc.inline_tensor(ident_np, name="ident128")
    dmask_np = np.kron(np.eye(32, dtype=np.float32), np.ones((4, 4), np.float32))
    dmask_d = nc.inline_tensor(dmask_np, name="blockdiag_mask")

    # row index = p*T + (tile offset) + k : partition p owns a contiguous span
    x_flat = x_d.ap().rearrange("(p r) e -> p (r e)", p=P)    # [128, T*20]
    l_flat = l_d.ap().rearrange("(p r) -> p r", p=P)          # [128, T]
    y_flat = y_d.ap().rearrange("(p r) e -> p (r e)", p=P)    # [128, T*4]

    with tile.TileContext(nc) as tc:
        with (
            tc.tile_pool(name="xin", bufs=2) as xin_pool,
            tc.tile_pool(name="lin", bufs=2) as lin_pool,
            tc.tile_pool(name="zp", bufs=2) as z_pool,
            tc.tile_pool(name="g", bufs=5) as g_pool,
            tc.tile_pool(name="stage", bufs=3) as stage_pool,
            tc.tile_pool(name="singles", bufs=1) as singles,
            tc.tile_pool(name="ps_v", bufs=2, space="PSUM") as ps_v,
            tc.tile_pool(name="ps_vt", bufs=2, space="PSUM") as ps_vt,
            tc.tile_pool(name="ps_y1", bufs=2, space="PSUM") as ps_y1,
            tc.tile_pool(name="ps_y2", bufs=2, space="PSUM") as ps_y2,
        ):
            # ---- one-time setup ----
            ident = singles.tile([128, 128], f32)
            nc.sync.dma_start(out=ident[:], in_=ident_d.ap())
            ident_r = singles.tile([128, 128], f32r)
            nc.vector.tensor_copy(ident_r[:], ident[:])
            dmask = singles.tile([128, 128], f32)
            nc.sync.dma_start(out=dmask[:], in_=dmask_d.ap())
            wk_s = singles.tile([4, 4], f32)
            nc.sync.dma_start(out=wk_s[:], in_=wk_d.ap())            # [c, f]
            wr_s = singles.tile([4, 4], f32)
            nc.sync.dma_start(out=wr_s[:], in_=wr_d.ap().transpose([1, 0]))

            # W_full[4a+f, 4b+m] = sum_c Wk[c,f] * Wreg[m,c] = A[m,f]
            wk_rep = bass.AP(tensor=wk_s.tensor, offset=wk_s.offset,
                             ap=[list(wk_s.ap[0]), [0, 32], [1, 4]])
            wr_rep = bass.AP(tensor=wr_s.tensor, offset=wr_s.offset,
                             ap=[list(wr_s.ap[0]), [0, 32], [1, 4]])
            wkr = singles.tile([4, 128], f32)
            nc.vector.tensor_copy(wkr[:], wk_rep)
            wrr = singles.tile([4, 128], f32)
            nc.vector.tensor_copy(wrr[:], wr_rep)
            wfull_ps = ps_v.tile([128, 128], f32, tag="v_ps")
            nc.tensor.matmul(wfull_ps[:], wkr[:], wrr[:])
            w_sb = singles.tile([128, 128], f32r)
            nc.vector.tensor_mul(w_sb[:], wfull_ps[:], dmask[:])

            # ---- main loop ----
            base = 0
            for ki, K in enumerate(k_tiles):
                xt = xin_pool.tile([P, K * 20], f32)
                nc.sync.dma_start(out=xt[:],
                                  in_=x_flat[:, base * 20:(base + K) * 20])
                lt = lin_pool.tile([P, K], i32)
                nc.sync.dma_start(out=lt[:], in_=l_flat[:, base:base + K])

                x4 = xt.rearrange("p (k s e) -> p k s e", s=4, e=5)

                # Z[s, k] = (L[k] > s) * q0[k, s]   (s-major dense rows)
                z = z_pool.tile([P, 4, K], f32, tag="z")
                for s in range(4):
                    nc.vector.scalar_tensor_tensor(
                        out=z[:, s, :],
                        in0=lt[:],
                        scalar=float(s),
                        in1=x4[:, :, s, 0],
                        op0=mybir.AluOpType.is_gt,
                        op1=mybir.AluOpType.mult,
                    )

                # M_s[k, f] = Z[s, k] * x[k, s, f]; s=0,1 on DVE, s=2,3 GpSimd
                ms = []
                for s in range(4):
                    m = g_pool.tile([P, K, 4], f32r, tag="g")
                    zb = z[:, s, :].unsqueeze(2).broadcast_to([P, K, 4])
                    eng = nc.vector if s < 2 else nc.gpsimd
                    eng.tensor_mul(m[:], zb, x4[:, :, s, 1:5])
                    ms.append(m.rearrange("p k f -> p (k f)"))

                n_groups = (K * 4) // GROUP
                for g in range(n_groups):
                    sl = slice(g * GROUP, (g + 1) * GROUP)
                    # v = sum_s M_s  (PE identity-accumulate, fp32r)
                    v_ps = ps_v.tile([128, GROUP], f32)
                    for s in range(4):
                        nc.tensor.matmul(v_ps[:], ident_r[:],
                                         ms[s][:, sl],
                                         start=(s == 0), stop=(s == 3))
                    v_sb = stage_pool.tile([128, GROUP], f32, tag="v")
                    nc.scalar.copy(v_sb[:], v_ps[:])

                    # vT: transpose each 128-col chunk  (fp32, exact)
                    vt_ps = ps_vt.tile([128, GROUP], f32)
                    for j in range(GROUP // 128):
                        cj = slice(j * 128, (j + 1) * 128)
                        nc.tensor.transpose(vt_ps[:, cj], v_sb[:, cj], ident[:])
                    vt_sb = stage_pool.tile([128, GROUP], f32r, tag="vt")
                    nc.scalar.copy(vt_sb[:], vt_ps[:])

                    # y' = W^T-blockdiag applied: y'[(4k+m), p-col]
                    y1_ps = ps_y1.tile([128, GROUP], f32)
                    nc.tensor.matmul(y1_ps[:], w_sb[:], vt_sb[:])
                    y1_sb = stage_pool.tile([128, GROUP], f32, tag="y1")
                    nc.scalar.copy(y1_sb[:], y1_ps[:])

                    # transpose back to sample-major, stage to SBUF, DMA out
                    y2_ps = ps_y2.tile([128, GROUP], f32)
                    for j in range(GROUP // 128):
                        cj = slice(j * 128, (j + 1) * 128)
                        nc.tensor.transpose(y2_ps[:, cj], y1_sb[:, cj], ident[:])
                    y2_sb = stage_pool.tile([128, GROUP], f32, tag="y2")
                    nc.scalar.copy(y2_sb[:], y2_ps[:])
                    col0 = (base + g * (GROUP // 4)) * 4
                    nc.sync.dma_start(out=y_flat[:, col0:col0 + GROUP],
                                      in_=y2_sb[:])
                base += K
    nc.compile()
    return nc, bs_pad


_NC_CACHE = None


def _get_nc():
    global _NC_CACHE
    if _NC_CACHE is None:
        _NC_CACHE = build_nc()
    return _NC_CACHE


def _shard_inputs(xss, seq_lengths, W_kernel, W_reg, bs_pad):
    x2 = np.ascontiguousarray(xss.reshape(B_TOTAL, 20), dtype=np.float32)
    seq = np.ascontiguousarray(seq_lengths, dtype=np.int32)
    wk = np.ascontiguousarray(W_kernel, dtype=np.float32)
    wr = np.ascontiguousarray(W_reg, dtype=np.float32)
    in_maps = []
    pad = bs_pad - BS
    for c in range(N_CORES):
        xs = x2[c * BS:(c + 1) * BS]
        ls = seq[c * BS:(c + 1) * BS]
        if pad:
            xs = np.concatenate([xs, np.zeros((pad, 20), np.float32)], axis=0)
            ls = np.concatenate([ls, np.zeros((pad,), np.int32)], axis=0)
        in_maps.append({"xss": xs, "seq": ls, "w_kernel": wk, "w_reg": wr})
    return in_maps


def run(xss, seq_lengths, W_kernel, W_reg, trace=False, **spmd_kwargs):
    nc, bs_pad = _get_nc()
    in_maps = _shard_inputs(xss, seq_lengths, W_kernel, W_reg, bs_pad)
    res = run_bass_kernel_spmd(nc, in_maps, core_ids=list(range(N_CORES)),
                               trace=trace, **spmd_kwargs)
    parts = [r["y"][:BS] for r in res.results]
    out = np.concatenate(parts, axis=0)
    return out, res


def kernel(xss, seq_lengths, W_kernel, W_reg):
    out, _ = run(xss, seq_lengths, W_kernel, W_reg)
    return out

